# revision 26
# baseline (speedup 1.0000x reference)
"""ArcFace (AngularPenaltySMLoss) over [32768, 8192] f32, distributed over
8 TRN2 NeuronCores, data-parallel on the batch dim.

Per core: shard [4096, 8192], reshaped host-side to [16, 128, 16384] so
each SBUF partition holds TWO consecutive rows (64KB contiguous per
partition per tile). HWDGE splits a transfer into contiguous
partition-blocks of count/16 per DMA engine (partition count must be a
multiple of 16 — a 127-row transfer degenerates to ONE engine), and the
ring-housekeeping engine E79 runs ~13% slower while busy, pacing the
stream. Two rows per partition halves the descriptor count per byte
(128 x 64KB vs 256 x 32KB per 256 rows), trimming E79's overhead.

Tiles alternate between the two HWDGE rings (sync/scalar): a single
ring's issue->completion-sem->next-issue chain adds ~5us per transfer,
pacing one ring at ~24.7us/tile vs the ~19.7us transfer itself.

Per [128, 16384] tile (one 8MB transfer; 16 tiles, no remainder):
  - ScalarE: exp(S*x) in four 4096-wide chunks (PSUM free-dim cap) with
    fused accumulation; chunks 0,1 are row 2p (-> col 2k), chunks 2,3
    row 2p+1 (-> col 2k+1).
  - GpSimd ap_gather on each 8192-col half: pulls x[p, lab[16*(p//16)+i]]
    into a [128,16] block (indices wrap per 16-partition group);
    VectorE scalar_tensor_tensor with a diagonal mask extracts
    x[p, lab[p]] -> tvals. Replaces the old iota==label scan that kept
    VectorE 84% busy and rate-matched with the DMA stream.
Last tile is fetched in four 4096-col transfers and its final chunk
exp'd in two 2048-wide pieces, so only ~2.5us of ScalarE work is
exposed after the final DMA byte; the epilogue for cols 0..29 hides
under the last tile's stream.
Epilogue:
  numerator = S*(t*cos(M) - sin(M)*sqrt(1 - t^2))   # = S*cos(acos(t)+M)
  with sqrt(y) = exp(0.5*ln(y)) so the only ACT table set used is
  natural_log_exp (zero mid-kernel table switches).
  L = numerator - log(exp(numerator) + rowsum - exp(S*t))
Final: GpSimd XYZWC-reduce of ell[128,32] to [1,1] and a 4-byte out-DMA
(a [128,1] out costs ~6us of trailing per-engine semaphore straggle).
"""

import numpy as np

from concourse import bacc, hw_specs, mybir, tile
from concourse.bass_utils import run_bass_kernel_spmd

# The act-table placement pass picks the FIRST set containing each
# activation function, so an Exp/Ln mix thrashes between exp_and_others and
# natural_log (8 table loads, 3 on the critical tail). Present a view
# of the tables with Exp/Ln stripped from every set except the combined
# natural_log_exp_and_others so both resolve to one set (one load total).
# Only membership changes; set order/ids still match act_info.json.
_ORIG_GET_TABLES = hw_specs.get_activation_tables
_COMBINED_SET = "natural_log_exp_and_others"


def _exp_ln_combined_tables(arch):
    tabs = _ORIG_GET_TABLES(arch)
    AF = mybir.ActivationFunctionType
    if _COMBINED_SET not in tabs:
        return tabs
    return {
        name: (fns - {AF.Exp, AF.Ln} if name != _COMBINED_SET else fns)
        for name, fns in tabs.items()
    }


N, C = 32768, 8192
N_CORES = 8
N_SHARD = N // N_CORES      # 4096 rows per core
P = 128                     # SBUF partitions
RPP = 2                     # rows per partition
TROWS = P * RPP             # 256 rows per tile
N_T = N_SHARD // TROWS      # 16 tiles per core
W = C * RPP                 # 16384 cols per tile
NCOL = N_T * RPP            # 32 per-row-state columns
S = 32.0
M = 0.5
EPS = 1e-7

_F32 = mybir.dt.float32
_I16 = mybir.dt.int16


def build(out_scalar=True, x_bufs=3):
    prev_tables = bacc.get_activation_tables
    bacc.get_activation_tables = _exp_ln_combined_tables
    try:
        return _build(out_scalar, x_bufs)
    finally:
        bacc.get_activation_tables = prev_tables


def _build(out_scalar, x_bufs):
    nc = bacc.Bacc(None, target_bir_lowering=False)

    x_ext = nc.declare_dram_parameter("cls_score", [N_T, P, W], _F32,
                                      isOutput=False)
    # gather-A index of tile k at col 4k, gather-B at col 4k+2: the
    # index operand needs a 4-byte-aligned SBUF offset
    lab_ext = nc.declare_dram_parameter("labels_t", [P, 4 * N_T], _I16,
                                        isOutput=False)
    diag_ext = nc.declare_dram_parameter("diag16", [P, 16], _F32,
                                         isOutput=False)
    out_shape = [1, 1] if out_scalar else [P, 1]
    out_ext = nc.declare_dram_parameter("out", out_shape, _F32, isOutput=True)

    AF = mybir.ActivationFunctionType
    OP = mybir.AluOpType
    AX = mybir.AxisListType

    CH = 4096               # exp chunk width (PSUM free-dim cap)

    with tile.TileContext(nc) as tc:
        with (
            tc.tile_pool(name="xp", bufs=x_bufs) as xp,
            tc.tile_pool(name="ep", bufs=1, space="PSUM") as ep,
            tc.tile_pool(name="mp", bufs=2) as mp,
            tc.tile_pool(name="st", bufs=1) as st,
        ):
            lab = st.tile([P, 4 * N_T], _I16)
            nc.scalar.dma_start(out=lab[:], in_=lab_ext[:])
            diag = st.tile([P, 16], _F32)
            nc.scalar.dma_start(out=diag[:], in_=diag_ext[:])

            sumexp = st.tile([P, NCOL], _F32)
            sumexpA = st.tile([P, NCOL], _F32)  # first-half chunk sums
            sumexpB = st.tile([P, NCOL], _F32)  # second-half chunk sums
            tailacc = st.tile([P, 2], _F32)     # last chunk's 2048-wide pair
            tvals = st.tile([P, NCOL], _F32)

            # epilogue scratch, written in column batches
            tclip = st.tile([P, NCOL], _F32)
            tsq = st.tile([P, NCOL], _F32)
            om = st.tile([P, NCOL], _F32)
            lnom = st.tile([P, NCOL], _F32)
            r = st.tile([P, NCOL], _F32)
            b_t = st.tile([P, NCOL], _F32)
            num = st.tile([P, NCOL], _F32)
            e_num = st.tile([P, NCOL], _F32)
            e_st = st.tile([P, NCOL], _F32)
            excl = st.tile([P, NCOL], _F32)
            denom = st.tile([P, NCOL], _F32)
            logd = st.tile([P, NCOL], _F32)
            ell = st.tile([P, NCOL], _F32)

            def epilogue(sl):
                # all [P, width] ops; only Exp/Ln on ACT (one table set)
                nc.vector.tensor_scalar(
                    tclip[:, sl], tvals[:, sl], -1.0 + EPS, 1.0 - EPS,
                    OP.max, OP.min)
                nc.vector.tensor_tensor(tsq[:, sl], tclip[:, sl],
                                        tclip[:, sl], OP.mult)
                nc.vector.tensor_scalar(om[:, sl], tsq[:, sl], -1.0, 1.0,
                                        OP.mult, OP.add)  # 1 - t^2
                nc.scalar.activation(out=lnom[:, sl], in_=om[:, sl],
                                     func=AF.Ln)
                nc.scalar.activation(out=r[:, sl], in_=lnom[:, sl],
                                     func=AF.Exp, scale=0.5)  # sqrt(1-t^2)
                nc.vector.tensor_scalar_mul(b_t[:, sl], r[:, sl],
                                            S * float(np.sin(M)))
                nc.vector.scalar_tensor_tensor(
                    num[:, sl], tclip[:, sl], S * float(np.cos(M)),
                    b_t[:, sl], OP.mult, OP.subtract)
                nc.scalar.activation(out=e_num[:, sl], in_=num[:, sl],
                                     func=AF.Exp)
                nc.scalar.activation(out=e_st[:, sl], in_=tvals[:, sl],
                                     func=AF.Exp, scale=S)
                nc.vector.scalar_tensor_tensor(
                    excl[:, sl], e_st[:, sl], -1.0, sumexp[:, sl],
                    OP.mult, OP.add)  # sumexp - exp(S t)
                nc.vector.tensor_tensor(denom[:, sl], excl[:, sl],
                                        e_num[:, sl], OP.add)
                nc.scalar.activation(out=logd[:, sl], in_=denom[:, sl],
                                     func=AF.Ln)
                nc.vector.tensor_tensor(ell[:, sl], num[:, sl], logd[:, sl],
                                        OP.subtract)

            def do_exp(xt, chunk_cs, acc_ap):
                et = ep.tile([P, chunk_cs.stop - chunk_cs.start], _F32)
                nc.scalar.activation(
                    out=et[:], in_=xt[:, chunk_cs], func=AF.Exp, scale=S,
                    accum_out=acc_ap,
                )

            def gather(xt, k, half_idx):
                # gather on one 8192-col half; plain labels index the half
                g = mp.tile([P, 16], _F32)
                cs = slice(half_idx * C, (half_idx + 1) * C)
                nc.gpsimd.ap_gather(g[:], xt[:, cs],
                                    lab[:, 4 * k + 2 * half_idx:
                                        4 * k + 2 * half_idx + 1],
                                    channels=P, num_elems=C, d=1, num_idxs=16)
                mt = mp.tile([P, 16], _F32)
                nc.vector.scalar_tensor_tensor(
                    mt[:], g[:], 1.0, diag[:], OP.mult, OP.mult,
                    accum_out=tvals[:, RPP * k + half_idx:
                                    RPP * k + half_idx + 1])

            acc_of = {0: sumexpA, 1: sumexpB, 2: sumexpA, 3: sumexpB}

            # Each tile is TWO half transfers on the two HWDGE rings:
            # [0:8192] on sync, [8192:16384] on scalar. A single ring's
            # issue->completion-sem->next-issue chain costs ~5us per
            # transfer (pacing one ring at ~24.7us/tile instead of
            # ~19.7), and free-running whole tiles on alternate rings
            # drifts out of order, leaving ScalarE (in-order consumer)
            # tiles of backlog at stream end. Half-per-ring keeps the
            # rings in lockstep and halves the data each sem gates.
            # Alternate tiles across BOTH HWDGE rings: a single ring's
            # issue->completion-sem->next-issue chain costs ~5us per
            # transfer on top of the transfer itself, pacing one ring at
            # ~24.7us/tile instead of ~19.7. With two rings the chain
            # latency hides under the other ring's transfer.
            for k in range(N_T - 1):
                ring = nc.sync if k % 2 == 0 else nc.scalar
                xt = xp.tile([P, W], _F32)
                if k == 0:
                    # split the first tile so ScalarE starts ~10us earlier
                    nc.sync.dma_start(out=xt[:, 0:2 * CH],
                                      in_=x_ext[k, :, 0:2 * CH])
                    nc.scalar.dma_start(out=xt[:, 2 * CH:W],
                                        in_=x_ext[k, :, 2 * CH:W])
                else:
                    ring.dma_start(out=xt[:], in_=x_ext[k, :, :])
                for q in range(4):
                    col = RPP * k + q // 2
                    do_exp(xt, slice(q * CH, (q + 1) * CH),
                           acc_of[q][:, col:col + 1])
                gather(xt, k, 0)
                gather(xt, k, 1)

            # last tile: a 12288-col transfer then two 2048-col ones, the
            # final chunk exp'd in 2048-wide pieces, so only ~2.6us of
            # ScalarE work trails the last DMA byte
            k = N_T - 1
            xt = xp.tile([P, W], _F32)
            nc.scalar.dma_start(out=xt[:, 0:3 * CH], in_=x_ext[k, :, 0:3 * CH])
            for q in range(3):
                col = RPP * k + q // 2
                do_exp(xt, slice(q * CH, (q + 1) * CH),
                       acc_of[q][:, col:col + 1])
            gather(xt, k, 0)
            # cols 0..29 complete; their epilogue hides under the last
            # tile's remaining stream
            nc.vector.tensor_tensor(
                sumexp[:, 0:NCOL - 2], sumexpA[:, 0:NCOL - 2],
                sumexpB[:, 0:NCOL - 2], OP.add)
            epilogue(slice(0, NCOL - 2))
            for h in range(2):
                cs = slice(3 * CH + h * (CH // 2), 3 * CH + (h + 1) * (CH // 2))
                nc.sync.dma_start(out=xt[:, cs], in_=x_ext[k, :, cs])
                do_exp(xt, cs, tailacc[:, h:h + 1])
            gather(xt, k, 1)
            nc.vector.tensor_reduce(sumexpB[:, NCOL - 1:NCOL], tailacc[:],
                                    axis=AX.X, op=OP.add)
            nc.vector.tensor_tensor(
                sumexp[:, NCOL - 2:NCOL], sumexpA[:, NCOL - 2:NCOL],
                sumexpB[:, NCOL - 2:NCOL], OP.add)
            epilogue(slice(NCOL - 2, NCOL))

            if out_scalar:
                osb = st.tile([1, 1], _F32)
                nc.gpsimd.tensor_reduce(osb[:], ell[:], axis=AX.XYZWC,
                                        op=OP.add)
                nc.sync.dma_start(out=out_ext[:], in_=osb[:])
            else:
                lrow = st.tile([P, 1], _F32)
                nc.vector.tensor_reduce(lrow[:], ell[:], axis=AX.X, op=OP.add)
                nc.sync.dma_start(out=out_ext[:], in_=lrow[:])

    nc.finalize()
    return nc


_NC_CACHE = {}


def _get_nc():
    if "nc" not in _NC_CACHE:
        _NC_CACHE["nc"] = build()
    return _NC_CACHE["nc"]


def make_in_maps(cls_score, labels):
    cls_score = np.ascontiguousarray(np.asarray(cls_score, dtype=np.float32))
    labels = np.asarray(labels).astype(np.int64)
    diag = np.zeros((P, 16), np.float32)
    diag[np.arange(P), np.arange(P) % 16] = 1.0
    in_maps = []
    for i in range(N_CORES):
        shard = cls_score[i * N_SHARD:(i + 1) * N_SHARD]
        li = labels[i * N_SHARD:(i + 1) * N_SHARD]
        # partition p of tile k holds rows k*256 + 2p, 2p+1
        x3 = shard.reshape(N_T, P, W)
        lr = li.reshape(N_T, P, RPP)          # [k, p, row-in-partition]
        lab16 = np.zeros((P, 4 * N_T), np.int16)
        lab16[:, 0::4] = lr[:, :, 0].T        # gather-A: even rows
        lab16[:, 2::4] = lr[:, :, 1].T        # gather-B: odd rows
        in_maps.append({
            "cls_score": x3,
            "labels_t": np.ascontiguousarray(lab16),
            "diag16": diag,
        })
    return in_maps


def kernel(cls_score, labels):
    nc = _get_nc()
    in_maps = make_in_maps(cls_score, labels)
    res = run_bass_kernel_spmd(nc, in_maps, core_ids=list(range(N_CORES)))
    total = np.sum(
        [r["out"].astype(np.float64).sum() for r in res.results]
    )
    return np.float32(-(total / N))


# revision 29
# speedup vs baseline: 1.0196x; 1.0196x over previous
"""ArcFace (AngularPenaltySMLoss) over [32768, 8192] f32, distributed over
8 TRN2 NeuronCores, data-parallel on the batch dim.

Per core: shard [4096, 8192], reshaped host-side to [16, 128, 16384] so
each SBUF partition holds TWO consecutive rows (64KB contiguous per
partition per tile). HWDGE splits a transfer into contiguous
partition-blocks of count/16 per DMA engine (partition count must be a
multiple of 16 — a 127-row transfer degenerates to ONE engine), and the
ring-housekeeping engine E79 runs ~13% slower while busy, pacing the
stream. Two rows per partition halves the descriptor count per byte
(128 x 64KB vs 256 x 32KB per 256 rows), trimming E79's overhead.

Each tile moves as TWO half transfers, [0:8192] on the sync ring and
[8192:16384] on the scalar ring: a single ring's issue->completion-sem->
next-issue chain adds ~5us per transfer (~24.7us/tile vs the ~19.7us
transfer itself), while whole tiles free-running on alternate rings
drift out of completion order and leave ScalarE (in-order consumer) a
backlog at stream end. Half-per-ring keeps both rings in lockstep.

Per [128, 16384] tile (one 8MB transfer; 16 tiles, no remainder):
  - ScalarE: exp(S*x) in four 4096-wide chunks (PSUM free-dim cap) with
    fused accumulation; chunks 0,1 are row 2p (-> col 2k), chunks 2,3
    row 2p+1 (-> col 2k+1).
  - GpSimd ap_gather on each 8192-col half: pulls x[p, lab[16*(p//16)+i]]
    into a [128,16] block (indices wrap per 16-partition group);
    VectorE scalar_tensor_tensor with a diagonal mask extracts
    x[p, lab[p]] -> tvals. Replaces the old iota==label scan that kept
    VectorE 84% busy and rate-matched with the DMA stream.
Last tile is fetched in four 4096-col transfers and its final chunk
exp'd in two 2048-wide pieces, so only ~2.5us of ScalarE work is
exposed after the final DMA byte; the epilogue for cols 0..29 hides
under the last tile's stream.
Epilogue:
  numerator = S*(t*cos(M) - sin(M)*sqrt(1 - t^2))   # = S*cos(acos(t)+M)
  with sqrt(y) = exp(0.5*ln(y)) so the only ACT table set used is
  natural_log_exp (zero mid-kernel table switches).
  L = numerator - log(exp(numerator) + rowsum - exp(S*t))
Final: GpSimd XYZWC-reduce of ell[128,32] to [1,1] and a 4-byte out-DMA
(a [128,1] out costs ~6us of trailing per-engine semaphore straggle).
"""

import numpy as np

from concourse import bacc, hw_specs, mybir, tile
from concourse.bass_utils import run_bass_kernel_spmd

# The act-table placement pass picks the FIRST set containing each
# activation function, so an Exp/Ln mix thrashes between exp_and_others and
# natural_log (8 table loads, 3 on the critical tail). Present a view
# of the tables with Exp/Ln stripped from every set except the combined
# natural_log_exp_and_others so both resolve to one set (one load total).
# Only membership changes; set order/ids still match act_info.json.
_ORIG_GET_TABLES = hw_specs.get_activation_tables
_COMBINED_SET = "natural_log_exp_and_others"


def _exp_ln_combined_tables(arch):
    tabs = _ORIG_GET_TABLES(arch)
    AF = mybir.ActivationFunctionType
    if _COMBINED_SET not in tabs:
        return tabs
    return {
        name: (fns - {AF.Exp, AF.Ln} if name != _COMBINED_SET else fns)
        for name, fns in tabs.items()
    }


N, C = 32768, 8192
N_CORES = 8
N_SHARD = N // N_CORES      # 4096 rows per core
P = 128                     # SBUF partitions
RPP = 2                     # rows per partition
TROWS = P * RPP             # 256 rows per tile
N_T = N_SHARD // TROWS      # 16 tiles per core
W = C * RPP                 # 16384 cols per tile
NCOL = N_T * RPP            # 32 per-row-state columns
S = 32.0
M = 0.5
EPS = 1e-7

_F32 = mybir.dt.float32
_I16 = mybir.dt.int16


def build(out_scalar=True, x_bufs=3):
    prev_tables = bacc.get_activation_tables
    bacc.get_activation_tables = _exp_ln_combined_tables
    try:
        return _build(out_scalar, x_bufs)
    finally:
        bacc.get_activation_tables = prev_tables


def _build(out_scalar, x_bufs):
    nc = bacc.Bacc(None, target_bir_lowering=False)

    x_ext = nc.declare_dram_parameter("cls_score", [N_T, P, W], _F32,
                                      isOutput=False)
    # gather-A index of tile k at col 4k, gather-B at col 4k+2: the
    # index operand needs a 4-byte-aligned SBUF offset
    lab_ext = nc.declare_dram_parameter("labels_t", [P, 4 * N_T], _I16,
                                        isOutput=False)
    diag_ext = nc.declare_dram_parameter("diag16", [P, 16], _F32,
                                         isOutput=False)
    out_shape = [1, 1] if out_scalar else [P, 1]
    out_ext = nc.declare_dram_parameter("out", out_shape, _F32, isOutput=True)

    AF = mybir.ActivationFunctionType
    OP = mybir.AluOpType
    AX = mybir.AxisListType

    CH = 4096               # exp chunk width (PSUM free-dim cap)

    with tile.TileContext(nc) as tc:
        with (
            tc.tile_pool(name="xp", bufs=x_bufs) as xp,
            tc.tile_pool(name="ep", bufs=1, space="PSUM") as ep,
            tc.tile_pool(name="mp", bufs=2) as mp,
            tc.tile_pool(name="st", bufs=1) as st,
        ):
            lab = st.tile([P, 4 * N_T], _I16)
            nc.scalar.dma_start(out=lab[:], in_=lab_ext[:])
            diag = st.tile([P, 16], _F32)
            nc.scalar.dma_start(out=diag[:], in_=diag_ext[:])

            sumexp = st.tile([P, NCOL], _F32)
            sumexpA = st.tile([P, NCOL], _F32)  # first-half chunk sums
            sumexpB = st.tile([P, NCOL], _F32)  # second-half chunk sums
            tailacc = st.tile([P, 2], _F32)     # last chunk's 2048-wide pair
            tvals = st.tile([P, NCOL], _F32)

            # epilogue scratch, written in column batches
            tclip = st.tile([P, NCOL], _F32)
            tsq = st.tile([P, NCOL], _F32)
            om = st.tile([P, NCOL], _F32)
            lnom = st.tile([P, NCOL], _F32)
            r = st.tile([P, NCOL], _F32)
            b_t = st.tile([P, NCOL], _F32)
            num = st.tile([P, NCOL], _F32)
            e_num = st.tile([P, NCOL], _F32)
            e_st = st.tile([P, NCOL], _F32)
            excl = st.tile([P, NCOL], _F32)
            denom = st.tile([P, NCOL], _F32)
            logd = st.tile([P, NCOL], _F32)
            ell = st.tile([P, NCOL], _F32)

            def epilogue(sl):
                # all [P, width] ops; only Exp/Ln on ACT (one table set)
                nc.vector.tensor_scalar(
                    tclip[:, sl], tvals[:, sl], -1.0 + EPS, 1.0 - EPS,
                    OP.max, OP.min)
                nc.vector.tensor_tensor(tsq[:, sl], tclip[:, sl],
                                        tclip[:, sl], OP.mult)
                nc.vector.tensor_scalar(om[:, sl], tsq[:, sl], -1.0, 1.0,
                                        OP.mult, OP.add)  # 1 - t^2
                nc.scalar.activation(out=lnom[:, sl], in_=om[:, sl],
                                     func=AF.Ln)
                nc.scalar.activation(out=r[:, sl], in_=lnom[:, sl],
                                     func=AF.Exp, scale=0.5)  # sqrt(1-t^2)
                nc.vector.tensor_scalar_mul(b_t[:, sl], r[:, sl],
                                            S * float(np.sin(M)))
                nc.vector.scalar_tensor_tensor(
                    num[:, sl], tclip[:, sl], S * float(np.cos(M)),
                    b_t[:, sl], OP.mult, OP.subtract)
                nc.scalar.activation(out=e_num[:, sl], in_=num[:, sl],
                                     func=AF.Exp)
                nc.scalar.activation(out=e_st[:, sl], in_=tvals[:, sl],
                                     func=AF.Exp, scale=S)
                nc.vector.scalar_tensor_tensor(
                    excl[:, sl], e_st[:, sl], -1.0, sumexp[:, sl],
                    OP.mult, OP.add)  # sumexp - exp(S t)
                nc.vector.tensor_tensor(denom[:, sl], excl[:, sl],
                                        e_num[:, sl], OP.add)
                nc.scalar.activation(out=logd[:, sl], in_=denom[:, sl],
                                     func=AF.Ln)
                nc.vector.tensor_tensor(ell[:, sl], num[:, sl], logd[:, sl],
                                        OP.subtract)

            def do_exp(xt, chunk_cs, acc_ap):
                et = ep.tile([P, chunk_cs.stop - chunk_cs.start], _F32)
                nc.scalar.activation(
                    out=et[:], in_=xt[:, chunk_cs], func=AF.Exp, scale=S,
                    accum_out=acc_ap,
                )

            def gather(xt, k, half_idx):
                # gather on one 8192-col half; plain labels index the half
                g = mp.tile([P, 16], _F32)
                cs = slice(half_idx * C, (half_idx + 1) * C)
                nc.gpsimd.ap_gather(g[:], xt[:, cs],
                                    lab[:, 4 * k + 2 * half_idx:
                                        4 * k + 2 * half_idx + 1],
                                    channels=P, num_elems=C, d=1, num_idxs=16)
                mt = mp.tile([P, 16], _F32)
                nc.vector.scalar_tensor_tensor(
                    mt[:], g[:], 1.0, diag[:], OP.mult, OP.mult,
                    accum_out=tvals[:, RPP * k + half_idx:
                                    RPP * k + half_idx + 1])

            acc_of = {0: sumexpA, 1: sumexpB, 2: sumexpA, 3: sumexpB}

            # Each tile is TWO half transfers on the two HWDGE rings:
            # [0:8192] on sync, [8192:16384] on scalar. A single ring's
            # issue->completion-sem->next-issue chain costs ~5us per
            # transfer (pacing one ring at ~24.7us/tile instead of
            # ~19.7), and free-running whole tiles on alternate rings
            # drifts out of order, leaving ScalarE (in-order consumer)
            # tiles of backlog at stream end. Half-per-ring keeps the
            # rings in lockstep and halves the data each sem gates.
            # Each tile is TWO half transfers on the two HWDGE rings:
            # [0:8192] on sync, [8192:16384] on scalar. A single ring's
            # issue->completion-sem->next-issue chain costs ~5us per
            # transfer (pacing one ring at ~24.7us/tile instead of
            # ~19.7), and free-running whole tiles on alternate rings
            # drift out of order, leaving ScalarE (in-order consumer)
            # tiles of backlog at stream end. Half-per-ring keeps the
            # rings in lockstep and halves the data each sem gates.
            for k in range(N_T - 1):
                xt = xp.tile([P, W], _F32)
                nc.sync.dma_start(out=xt[:, 0:2 * CH],
                                  in_=x_ext[k, :, 0:2 * CH])
                nc.scalar.dma_start(out=xt[:, 2 * CH:W],
                                    in_=x_ext[k, :, 2 * CH:W])
                for q in range(4):
                    col = RPP * k + q // 2
                    do_exp(xt, slice(q * CH, (q + 1) * CH),
                           acc_of[q][:, col:col + 1])
                gather(xt, k, 0)
                gather(xt, k, 1)

            # last tile: a 12288-col transfer then two 2048-col ones, the
            # final chunk exp'd in 2048-wide pieces, so only ~2.6us of
            # ScalarE work trails the last DMA byte
            k = N_T - 1
            xt = xp.tile([P, W], _F32)
            nc.sync.dma_start(out=xt[:, 0:2 * CH], in_=x_ext[k, :, 0:2 * CH])
            nc.scalar.dma_start(out=xt[:, 2 * CH:3 * CH],
                                in_=x_ext[k, :, 2 * CH:3 * CH])
            for q in range(3):
                col = RPP * k + q // 2
                do_exp(xt, slice(q * CH, (q + 1) * CH),
                       acc_of[q][:, col:col + 1])
            gather(xt, k, 0)
            # cols 0..29 complete; their epilogue hides under the last
            # tile's remaining stream
            nc.vector.tensor_tensor(
                sumexp[:, 0:NCOL - 2], sumexpA[:, 0:NCOL - 2],
                sumexpB[:, 0:NCOL - 2], OP.add)
            epilogue(slice(0, NCOL - 2))
            for h in range(2):
                cs = slice(3 * CH + h * (CH // 2), 3 * CH + (h + 1) * (CH // 2))
                ring = nc.sync if h == 0 else nc.scalar
                ring.dma_start(out=xt[:, cs], in_=x_ext[k, :, cs])
                do_exp(xt, cs, tailacc[:, h:h + 1])
            gather(xt, k, 1)
            nc.vector.tensor_reduce(sumexpB[:, NCOL - 1:NCOL], tailacc[:],
                                    axis=AX.X, op=OP.add)
            nc.vector.tensor_tensor(
                sumexp[:, NCOL - 2:NCOL], sumexpA[:, NCOL - 2:NCOL],
                sumexpB[:, NCOL - 2:NCOL], OP.add)
            epilogue(slice(NCOL - 2, NCOL))

            if out_scalar:
                osb = st.tile([1, 1], _F32)
                nc.gpsimd.tensor_reduce(osb[:], ell[:], axis=AX.XYZWC,
                                        op=OP.add)
                nc.sync.dma_start(out=out_ext[:], in_=osb[:])
            else:
                lrow = st.tile([P, 1], _F32)
                nc.vector.tensor_reduce(lrow[:], ell[:], axis=AX.X, op=OP.add)
                nc.sync.dma_start(out=out_ext[:], in_=lrow[:])

    nc.finalize()
    return nc


_NC_CACHE = {}


def _get_nc():
    if "nc" not in _NC_CACHE:
        _NC_CACHE["nc"] = build()
    return _NC_CACHE["nc"]


def make_in_maps(cls_score, labels):
    cls_score = np.ascontiguousarray(np.asarray(cls_score, dtype=np.float32))
    labels = np.asarray(labels).astype(np.int64)
    diag = np.zeros((P, 16), np.float32)
    diag[np.arange(P), np.arange(P) % 16] = 1.0
    in_maps = []
    for i in range(N_CORES):
        shard = cls_score[i * N_SHARD:(i + 1) * N_SHARD]
        li = labels[i * N_SHARD:(i + 1) * N_SHARD]
        # partition p of tile k holds rows k*256 + 2p, 2p+1
        x3 = shard.reshape(N_T, P, W)
        lr = li.reshape(N_T, P, RPP)          # [k, p, row-in-partition]
        lab16 = np.zeros((P, 4 * N_T), np.int16)
        lab16[:, 0::4] = lr[:, :, 0].T        # gather-A: even rows
        lab16[:, 2::4] = lr[:, :, 1].T        # gather-B: odd rows
        in_maps.append({
            "cls_score": x3,
            "labels_t": np.ascontiguousarray(lab16),
            "diag16": diag,
        })
    return in_maps


def kernel(cls_score, labels):
    nc = _get_nc()
    in_maps = make_in_maps(cls_score, labels)
    res = run_bass_kernel_spmd(nc, in_maps, core_ids=list(range(N_CORES)))
    total = np.sum(
        [r["out"].astype(np.float64).sum() for r in res.results]
    )
    return np.float32(-(total / N))


# revision 30
# speedup vs baseline: 1.2490x; 1.2250x over previous
"""ArcFace (AngularPenaltySMLoss) over [32768, 8192] f32, distributed over
8 TRN2 NeuronCores, data-parallel on the batch dim.

Per core: shard [4096, 8192], reshaped host-side to [16, 128, 16384] so
each SBUF partition holds TWO consecutive rows (64KB contiguous per
partition per tile). HWDGE splits a transfer into contiguous
partition-blocks of count/16 per DMA engine (partition count must be a
multiple of 16 — a 127-row transfer degenerates to ONE engine), and the
ring-housekeeping engine E79 runs ~13% slower while busy, pacing the
stream. Two rows per partition halves the descriptor count per byte
(128 x 64KB vs 256 x 32KB per 256 rows), trimming E79's overhead.

Each tile moves as TWO half transfers, [0:8192] on the sync ring and
[8192:16384] on the scalar ring: a single ring's issue->completion-sem->
next-issue chain adds ~5us per transfer (~24.7us/tile vs the ~19.7us
transfer itself), while whole tiles free-running on alternate rings
drift out of completion order and leave ScalarE (in-order consumer) a
backlog at stream end. Half-per-ring keeps both rings in lockstep.

Per [128, 16384] tile (one 8MB transfer; 16 tiles, no remainder):
  - ScalarE: exp(S*x) in four 4096-wide chunks (PSUM free-dim cap) with
    fused accumulation; chunks 0,1 are row 2p (-> col 2k), chunks 2,3
    row 2p+1 (-> col 2k+1).
  - GpSimd ap_gather on each 8192-col half: pulls x[p, lab[16*(p//16)+i]]
    into a [128,16] block (indices wrap per 16-partition group);
    VectorE scalar_tensor_tensor with a diagonal mask extracts
    x[p, lab[p]] -> tvals. Replaces the old iota==label scan that kept
    VectorE 84% busy and rate-matched with the DMA stream.
Last tile is fetched in four 4096-col transfers and its final chunk
exp'd in two 2048-wide pieces, so only ~2.5us of ScalarE work is
exposed after the final DMA byte; the epilogue for cols 0..29 hides
under the last tile's stream.
Epilogue:
  numerator = S*(t*cos(M) - sin(M)*sqrt(1 - t^2))   # = S*cos(acos(t)+M)
  with sqrt(y) = exp(0.5*ln(y)) so the only ACT table set used is
  natural_log_exp (zero mid-kernel table switches).
  L = numerator - log(exp(numerator) + rowsum - exp(S*t))
Final: GpSimd XYZWC-reduce of ell[128,32] to [1,1] and a 4-byte out-DMA
(a [128,1] out costs ~6us of trailing per-engine semaphore straggle).
"""

import numpy as np

from concourse import bacc, hw_specs, mybir, tile
from concourse.bass_utils import run_bass_kernel_spmd

# The act-table placement pass picks the FIRST set containing each
# activation function, so an Exp/Ln mix thrashes between exp_and_others and
# natural_log (8 table loads, 3 on the critical tail). Present a view
# of the tables with Exp/Ln stripped from every set except the combined
# natural_log_exp_and_others so both resolve to one set (one load total).
# Only membership changes; set order/ids still match act_info.json.
_ORIG_GET_TABLES = hw_specs.get_activation_tables
_COMBINED_SET = "natural_log_exp_and_others"


def _exp_ln_combined_tables(arch):
    tabs = _ORIG_GET_TABLES(arch)
    AF = mybir.ActivationFunctionType
    if _COMBINED_SET not in tabs:
        return tabs
    return {
        name: (fns - {AF.Exp, AF.Ln} if name != _COMBINED_SET else fns)
        for name, fns in tabs.items()
    }


N, C = 32768, 8192
N_CORES = 8
N_SHARD = N // N_CORES      # 4096 rows per core
P = 128                     # SBUF partitions
RPP = 2                     # rows per partition
TROWS = P * RPP             # 256 rows per tile
N_T = N_SHARD // TROWS      # 16 tiles per core
W = C * RPP                 # 16384 cols per tile
NCOL = N_T * RPP            # 32 per-row-state columns
S = 32.0
M = 0.5
EPS = 1e-7

_F32 = mybir.dt.float32
_I16 = mybir.dt.int16


def build(out_scalar=True, x_bufs=3):
    prev_tables = bacc.get_activation_tables
    bacc.get_activation_tables = _exp_ln_combined_tables
    try:
        return _build(out_scalar, x_bufs)
    finally:
        bacc.get_activation_tables = prev_tables


def _build(out_scalar, x_bufs):
    nc = bacc.Bacc(None, target_bir_lowering=False)

    x_ext = nc.declare_dram_parameter("cls_score", [N_T, P, W], _F32,
                                      isOutput=False)
    # gather-A index of tile k at col 4k, gather-B at col 4k+2: the
    # index operand needs a 4-byte-aligned SBUF offset
    lab_ext = nc.declare_dram_parameter("labels_t", [P, 4 * N_T], _I16,
                                        isOutput=False)
    diag_ext = nc.declare_dram_parameter("diag16", [P, 16], _F32,
                                         isOutput=False)
    out_shape = [1, 1] if out_scalar else [P, 1]
    out_ext = nc.declare_dram_parameter("out", out_shape, _F32, isOutput=True)

    AF = mybir.ActivationFunctionType
    OP = mybir.AluOpType
    AX = mybir.AxisListType

    CH = 4096               # exp chunk width (PSUM free-dim cap)

    with tile.TileContext(nc) as tc:
        with (
            tc.tile_pool(name="xp", bufs=x_bufs) as xp,
            tc.tile_pool(name="ep", bufs=1, space="PSUM") as ep,
            tc.tile_pool(name="mp", bufs=2) as mp,
            tc.tile_pool(name="st", bufs=1) as st,
        ):
            lab = st.tile([P, 4 * N_T], _I16)
            nc.scalar.dma_start(out=lab[:], in_=lab_ext[:])
            diag = st.tile([P, 16], _F32)
            nc.scalar.dma_start(out=diag[:], in_=diag_ext[:])

            sumexp = st.tile([P, NCOL], _F32)
            sumexpA = st.tile([P, NCOL], _F32)  # first-half chunk sums
            sumexpB = st.tile([P, NCOL], _F32)  # second-half chunk sums
            tailacc = st.tile([P, 3], _F32)     # last chunk's tail pieces
            tvals = st.tile([P, NCOL], _F32)

            # epilogue scratch, written in column batches
            tclip = st.tile([P, NCOL], _F32)
            tsq = st.tile([P, NCOL], _F32)
            om = st.tile([P, NCOL], _F32)
            lnom = st.tile([P, NCOL], _F32)
            r = st.tile([P, NCOL], _F32)
            b_t = st.tile([P, NCOL], _F32)
            num = st.tile([P, NCOL], _F32)
            e_num = st.tile([P, NCOL], _F32)
            e_st = st.tile([P, NCOL], _F32)
            excl = st.tile([P, NCOL], _F32)
            denom = st.tile([P, NCOL], _F32)
            logd = st.tile([P, NCOL], _F32)
            ell = st.tile([P, NCOL], _F32)

            def epilogue(sl):
                # all [P, width] ops; only Exp/Ln on ACT (one table set)
                nc.vector.tensor_scalar(
                    tclip[:, sl], tvals[:, sl], -1.0 + EPS, 1.0 - EPS,
                    OP.max, OP.min)
                nc.vector.tensor_tensor(tsq[:, sl], tclip[:, sl],
                                        tclip[:, sl], OP.mult)
                nc.vector.tensor_scalar(om[:, sl], tsq[:, sl], -1.0, 1.0,
                                        OP.mult, OP.add)  # 1 - t^2
                nc.scalar.activation(out=lnom[:, sl], in_=om[:, sl],
                                     func=AF.Ln)
                nc.scalar.activation(out=r[:, sl], in_=lnom[:, sl],
                                     func=AF.Exp, scale=0.5)  # sqrt(1-t^2)
                nc.vector.tensor_scalar_mul(b_t[:, sl], r[:, sl],
                                            S * float(np.sin(M)))
                nc.vector.scalar_tensor_tensor(
                    num[:, sl], tclip[:, sl], S * float(np.cos(M)),
                    b_t[:, sl], OP.mult, OP.subtract)
                nc.scalar.activation(out=e_num[:, sl], in_=num[:, sl],
                                     func=AF.Exp)
                nc.scalar.activation(out=e_st[:, sl], in_=tvals[:, sl],
                                     func=AF.Exp, scale=S)
                nc.vector.scalar_tensor_tensor(
                    excl[:, sl], e_st[:, sl], -1.0, sumexp[:, sl],
                    OP.mult, OP.add)  # sumexp - exp(S t)
                nc.vector.tensor_tensor(denom[:, sl], excl[:, sl],
                                        e_num[:, sl], OP.add)
                nc.scalar.activation(out=logd[:, sl], in_=denom[:, sl],
                                     func=AF.Ln)
                nc.vector.tensor_tensor(ell[:, sl], num[:, sl], logd[:, sl],
                                        OP.subtract)

            def do_exp(xt, chunk_cs, acc_ap):
                et = ep.tile([P, chunk_cs.stop - chunk_cs.start], _F32)
                nc.scalar.activation(
                    out=et[:], in_=xt[:, chunk_cs], func=AF.Exp, scale=S,
                    accum_out=acc_ap,
                )

            def gather(xt, k, half_idx):
                # gather on one 8192-col half; plain labels index the half
                g = mp.tile([P, 16], _F32)
                cs = slice(half_idx * C, (half_idx + 1) * C)
                nc.gpsimd.ap_gather(g[:], xt[:, cs],
                                    lab[:, 4 * k + 2 * half_idx:
                                        4 * k + 2 * half_idx + 1],
                                    channels=P, num_elems=C, d=1, num_idxs=16)
                mt = mp.tile([P, 16], _F32)
                nc.vector.scalar_tensor_tensor(
                    mt[:], g[:], 1.0, diag[:], OP.mult, OP.mult,
                    accum_out=tvals[:, RPP * k + half_idx:
                                    RPP * k + half_idx + 1])

            acc_of = {0: sumexpA, 1: sumexpB, 2: sumexpA, 3: sumexpB}

            # Each tile is TWO half transfers on the two HWDGE rings:
            # [0:8192] on sync, [8192:16384] on scalar. A single ring's
            # issue->completion-sem->next-issue chain costs ~5us per
            # transfer (pacing one ring at ~24.7us/tile instead of
            # ~19.7), and free-running whole tiles on alternate rings
            # drifts out of order, leaving ScalarE (in-order consumer)
            # tiles of backlog at stream end. Half-per-ring keeps the
            # rings in lockstep and halves the data each sem gates.
            for k in range(N_T - 1):
                xt = xp.tile([P, W], _F32)
                nc.sync.dma_start(out=xt[:, 0:2 * CH],
                                  in_=x_ext[k, :, 0:2 * CH])
                nc.scalar.dma_start(out=xt[:, 2 * CH:W],
                                    in_=x_ext[k, :, 2 * CH:W])
                for q in range(4):
                    col = RPP * k + q // 2
                    do_exp(xt, slice(q * CH, (q + 1) * CH),
                           acc_of[q][:, col:col + 1])
                gather(xt, k, 0)
                gather(xt, k, 1)

            # last tile: its sync-half plus [8192:12288] land like any
            # other tile; col 30 (even rows) is then complete, so the
            # epilogue for cols 0..30 hides under the remaining stream.
            # The final 4096 cols arrive as 2048+1024+1024 pieces so only
            # ~1.4us of ScalarE work trails the last DMA byte, and just
            # col 31's epilogue remains on the critical tail.
            k = N_T - 1
            xt = xp.tile([P, W], _F32)
            nc.sync.dma_start(out=xt[:, 0:2 * CH], in_=x_ext[k, :, 0:2 * CH])
            nc.scalar.dma_start(out=xt[:, 2 * CH:3 * CH],
                                in_=x_ext[k, :, 2 * CH:3 * CH])
            for q in range(3):
                col = RPP * k + q // 2
                do_exp(xt, slice(q * CH, (q + 1) * CH),
                       acc_of[q][:, col:col + 1])
            gather(xt, k, 0)
            nc.vector.tensor_tensor(
                sumexp[:, 0:NCOL - 1], sumexpA[:, 0:NCOL - 1],
                sumexpB[:, 0:NCOL - 1], OP.add)
            epilogue(slice(0, NCOL - 1))
            pieces = (slice(3 * CH, 3 * CH + CH // 2),
                      slice(3 * CH + CH // 2, 3 * CH + 3 * CH // 4),
                      slice(3 * CH + 3 * CH // 4, W))
            for h, cs in enumerate(pieces):
                ring = nc.sync if h % 2 == 0 else nc.scalar
                ring.dma_start(out=xt[:, cs], in_=x_ext[k, :, cs])
                do_exp(xt, cs, tailacc[:, h:h + 1])
            gather(xt, k, 1)
            nc.vector.tensor_reduce(sumexpB[:, NCOL - 1:NCOL], tailacc[:],
                                    axis=AX.X, op=OP.add)
            nc.vector.tensor_tensor(
                sumexp[:, NCOL - 1:NCOL], sumexpA[:, NCOL - 1:NCOL],
                sumexpB[:, NCOL - 1:NCOL], OP.add)
            epilogue(slice(NCOL - 1, NCOL))

            if out_scalar:
                osb = st.tile([1, 1], _F32)
                nc.gpsimd.tensor_reduce(osb[:], ell[:], axis=AX.XYZWC,
                                        op=OP.add)
                nc.sync.dma_start(out=out_ext[:], in_=osb[:])
            else:
                lrow = st.tile([P, 1], _F32)
                nc.vector.tensor_reduce(lrow[:], ell[:], axis=AX.X, op=OP.add)
                nc.sync.dma_start(out=out_ext[:], in_=lrow[:])

    nc.finalize()
    return nc


_NC_CACHE = {}


def _get_nc():
    if "nc" not in _NC_CACHE:
        _NC_CACHE["nc"] = build()
    return _NC_CACHE["nc"]


def make_in_maps(cls_score, labels):
    cls_score = np.ascontiguousarray(np.asarray(cls_score, dtype=np.float32))
    labels = np.asarray(labels).astype(np.int64)
    diag = np.zeros((P, 16), np.float32)
    diag[np.arange(P), np.arange(P) % 16] = 1.0
    in_maps = []
    for i in range(N_CORES):
        shard = cls_score[i * N_SHARD:(i + 1) * N_SHARD]
        li = labels[i * N_SHARD:(i + 1) * N_SHARD]
        # partition p of tile k holds rows k*256 + 2p, 2p+1
        x3 = shard.reshape(N_T, P, W)
        lr = li.reshape(N_T, P, RPP)          # [k, p, row-in-partition]
        lab16 = np.zeros((P, 4 * N_T), np.int16)
        lab16[:, 0::4] = lr[:, :, 0].T        # gather-A: even rows
        lab16[:, 2::4] = lr[:, :, 1].T        # gather-B: odd rows
        in_maps.append({
            "cls_score": x3,
            "labels_t": np.ascontiguousarray(lab16),
            "diag16": diag,
        })
    return in_maps


def kernel(cls_score, labels):
    nc = _get_nc()
    in_maps = make_in_maps(cls_score, labels)
    res = run_bass_kernel_spmd(nc, in_maps, core_ids=list(range(N_CORES)))
    total = np.sum(
        [r["out"].astype(np.float64).sum() for r in res.results]
    )
    return np.float32(-(total / N))
